# revision 38
# baseline (speedup 1.0000x reference)
"""Cross-attention kernel for Trainium2, 8 NeuronCores.

Sharding: batch (4) x head-group (2) = 8 cores. Each core computes, for its
batch b and its 8 heads: Q/K/V projections, softmax attention, and a partial
out-projection (row-parallel Wo). Host sums the two head-group partials per
batch and adds the bias (the "all-reduce after out_proj" done at unshard).

Device schedule (per core): two query chunks of 1024; per chunk, per head,
16 kpos-tiles of (scoreT matmuls -> exp -> PV matmuls). All projection
matmuls are drained from a background queue into the attention stream so
TensorE never idles; the chunk-0 out-projection runs under chunk-1's
attention. Softmax denominators ride as a ones-column in the PV matmul
(row 64 of the PSUM accumulator); normalization is DVE reciprocal ->
GpSimd partition-broadcast -> one fused DVE multiply per head.
"""

import numpy as np
import ml_dtypes

import concourse.bacc as bacc
import concourse.mybir as mybir
import concourse.tile as tile
from concourse.bass_utils import run_bass_kernel_spmd

BF16 = mybir.dt.bfloat16
F32 = mybir.dt.float32

B, S, D = 4, 2048, 1024
H_TOT, DH = 16, 64
H = 8                      # heads per core
DG = H * DH                # 512, head-group width
N_CORES = 8
P = 128
CH = 1024                  # query chunk width
NK = S // P                # 16 kpos tiles

_CACHED_NC = None
LAST_RESULT = None


def _emit_kernel():
    nc = bacc.Bacc()
    xT = nc.dram_tensor("xT", [D, S], BF16, kind="ExternalInput")
    cT = nc.dram_tensor("cT", [D, S], BF16, kind="ExternalInput")
    wq = nc.dram_tensor("wq", [D, DG], BF16, kind="ExternalInput")
    wk = nc.dram_tensor("wk", [D, DG], BF16, kind="ExternalInput")
    wv = nc.dram_tensor("wv", [D, DG], BF16, kind="ExternalInput")
    wo = nc.dram_tensor("wo", [DG, D], BF16, kind="ExternalInput")
    outT = nc.dram_tensor("outT", [D, S], BF16, kind="ExternalOutput")

    Exp = mybir.ActivationFunctionType.Exp

    with tile.TileContext(nc) as tc:
        with tc.tile_pool(name="big", bufs=1) as big, \
             tc.tile_pool(name="ptp", bufs=1) as ptp, \
             tc.tile_pool(name="nrm", bufs=1) as nrm, \
             tc.tile_pool(name="osg", bufs=3) as osg, \
             tc.tile_pool(name="ps", bufs=1, space="PSUM") as ps:

            # ---- resident tiles ----
            ct_sb = big.tile([P, 8, S], BF16, tag="ct")
            xt_sb = big.tile([P, 8, S], BF16, tag="xt")
            wq_sb = big.tile([P, 8, DG], BF16, tag="wq")
            wk_sb = big.tile([P, 8, DG], BF16, tag="wk")
            wv_sb = big.tile([P, 8, DG], BF16, tag="wv")
            wo_sb = big.tile([P, 4, D], BF16, tag="wo")
            kt = [big.tile([P, S], BF16, tag=f"kt{m}", name=f"kt{m}") for m in range(4)]
            qt = [big.tile([P, S], BF16, tag=f"qt{m}", name=f"qt{m}") for m in range(4)]
            vt = [big.tile([P, H, 66], BF16, tag=f"v{k}", name=f"v{k}") for k in range(NK)]
            atp = [[big.tile([P, CH], BF16, tag=f"atp{c}{m}", name=f"atp{c}{m}")
                    for m in range(4)] for c in range(2)]

            # input DMAs, split + ordered so consumers start as chunks land
            # (prelude-critical transfers in 256-col pieces: lower latency)
            for a in range(8):
                for pc in range(2):
                    ps_ = slice(pc * 256, (pc + 1) * 256)
                    nc.sync.dma_start(wk_sb[:, a, ps_],
                                      wk[a * P:(a + 1) * P, ps_])
                    nc.sync.dma_start(ct_sb[:, a, ps_],
                                      cT[a * P:(a + 1) * P, ps_])
            for a in range(8):
                for pc in range(2):
                    ps_ = slice(pc * 256, (pc + 1) * 256)
                    nc.sync.dma_start(wq_sb[:, a, ps_],
                                      wq[a * P:(a + 1) * P, ps_])
                    nc.sync.dma_start(xt_sb[:, a, ps_],
                                      xT[a * P:(a + 1) * P, ps_])
                for pc in range(2, 4):
                    ps_ = slice(pc * 256, (pc + 1) * 256)
                    nc.sync.dma_start(xt_sb[:, a, ps_],
                                      xT[a * P:(a + 1) * P, ps_])
            nc.sync.dma_start(wv_sb[:], wv[:].rearrange("(a p) n -> p a n", p=P))
            for qc in range(1, 4):
                cs = slice(qc * 512, (qc + 1) * 512)
                for a in range(8):
                    nc.sync.dma_start(ct_sb[:, a, cs], cT[a * P:(a + 1) * P, cs])
            nc.sync.dma_start(wo_sb[:], wo[:].rearrange("(a p) n -> p a n", p=P))
            for a in range(8):
                nc.sync.dma_start(xt_sb[:, a, CH:], xT[a * P:(a + 1) * P, CH:])

            # ---- projection emitters (one 512-col PSUM chunk each) ----
            def kproj(m, qc):
                acc = ps.tile([P, 512], F32, tag="pj", bufs=2)
                for a in range(8):
                    nc.tensor.matmul(
                        acc[:], wk_sb[:, a, m * P:(m + 1) * P],
                        ct_sb[:, a, qc * 512:(qc + 1) * 512],
                        start=(a == 0), stop=(a == 7))
                nc.vector.tensor_copy(kt[m][:, qc * 512:(qc + 1) * 512], acc[:])

            def qproj(m, qc):
                acc = ps.tile([P, 512], F32, tag="pj", bufs=2)
                for a in range(8):
                    nc.tensor.matmul(
                        acc[:], wq_sb[:, a, m * P:(m + 1) * P],
                        xt_sb[:, a, qc * 512:(qc + 1) * 512],
                        start=(a == 0), stop=(a == 7))
                nc.vector.tensor_copy(qt[m][:, qc * 512:(qc + 1) * 512], acc[:])

            def vproj(k):
                acc = ps.tile([P, 512], F32, tag="pj", bufs=2)
                for a in range(8):
                    nc.tensor.matmul(
                        acc[:], ct_sb[:, a, k * P:(k + 1) * P],
                        wv_sb[:, a, :],
                        start=(a == 0), stop=(a == 7))
                nc.vector.tensor_copy(
                    vt[k][:, :, 0:64], acc[:].rearrange("p (h d) -> p h d", h=H))
                nc.vector.memset(vt[k][:, :, 64:65], 1.0)

            def oproj(mt, qc):
                # one 512-col chunk of the out-projection, qc in 0..3
                c, q2 = qc // 2, qc % 2
                acc = ps.tile([P, 512], F32, tag="pj", bufs=2)
                for p_ in range(4):
                    nc.tensor.matmul(
                        acc[:], wo_sb[:, p_, mt * P:(mt + 1) * P],
                        atp[c][p_][:, q2 * 512:(q2 + 1) * 512],
                        start=(p_ == 0), stop=(p_ == 3))
                o_sl = osg.tile([P, 512], BF16, tag="ostage")
                nc.vector.tensor_copy(o_sl[:], acc[:])
                nc.sync.dma_start(
                    outT[mt * P:(mt + 1) * P, qc * 512:(qc + 1) * 512], o_sl[:])

            # background work queue: (release_du, emit_fn) — statically
            # scheduled so projection work spreads evenly (no bursts) while
            # meeting each consumer's deadline
            bg = []

            def drain_tick(u):
                while bg and bg[0][0] <= u:
                    bg.pop(0)[1]()

            def drain_all():
                while bg:
                    bg.pop(0)[1]()

            # ---- prelude: the bare minimum before attention can start ----
            # everything else streams in as released background work, so the
            # exp pipeline starts ~20us earlier while inputs still arrive
            kproj(0, 0)
            qproj(0, 0)
            qproj(0, 1)
            vproj(0)
            vproj(1)

            # static releases, arrival-aware (DMA order above) and spread to
            # meet each consumer: head h consumes m=h//2 from du 8h (chunk
            # 0) and du 64+8p (chunk 1, odd-first)
            sched = []
            sched.append((0, lambda: kproj(0, 1)))
            sched.append((2, lambda: kproj(0, 2)))
            sched.append((4, lambda: kproj(0, 3)))
            for k in range(2, NK):
                sched.append((max(0, k // 2 - 1), (lambda k=k: vproj(k))))
            for i, qc in enumerate(range(4)):
                sched.append((8 + 2 * i, (lambda qc=qc: kproj(1, qc))))
            sched.append((14, lambda: qproj(1, 0)))
            sched.append((15, lambda: qproj(1, 1)))
            rel = iter([18, 20, 22, 24, 26, 28,         # K/Q m2 by du 32
                        32, 34, 36, 38, 40, 42,         # K/Q m3 by du 48
                        46, 48, 50, 52,                 # Qc1 m0,m1 by du 64/72
                        54, 56, 58, 60])                # Qc1 m2,m3 by du 80/88
            for m in range(2, 4):
                for qc in range(4):
                    sched.append((next(rel), (lambda m=m, qc=qc: kproj(m, qc))))
                for qc in range(2):
                    sched.append((next(rel), (lambda m=m, qc=qc: qproj(m, qc))))
            for m in range(4):
                for qc in range(2, 4):
                    sched.append((next(rel), (lambda m=m, qc=qc: qproj(m, qc))))
            sched.sort(key=lambda t: t[0])
            bg.extend(sched)

            def norm_head(chunk, h, acc):
                # drain acc to SBUF fast (frees the PSUM bank), then
                # normalize off the critical path: recip -> p0 -> bcast -> mul
                m, rh = h // 2, 64 * (h % 2)
                usb = nrm.tile([65, CH], F32, tag="unm", bufs=2, name="unm")
                rb0 = nrm.tile([1, CH], F32, tag="rb0", bufs=2, name="rb0")
                bcb = nrm.tile([64, CH], F32, tag="bc", bufs=2, name="bc")
                stg = None
                if rh != 0:
                    stg = nrm.tile([64, CH], BF16, tag="stg", bufs=2, name="stg")
                for hf in range(2):
                    fs = slice(hf * 512, (hf + 1) * 512)
                    nc.vector.tensor_copy(usb[:, fs], acc[0:65, fs])
                    nc.sync.dma_start(rb0[0:1, fs], usb[64:65, fs])
                    nc.gpsimd.partition_broadcast(bcb[:, fs], rb0[0:1, fs])
                    # recip + the mul are both DVE: program order protects
                    # the in-place custom-op write
                    nc.vector.reciprocal_approx_fast(bcb[:, fs], bcb[:, fs])
                    if rh == 0:
                        nc.vector.tensor_mul(
                            atp[chunk][m][0:64, fs], usb[0:64, fs], bcb[:, fs])
                    else:
                        nc.vector.tensor_mul(stg[:, fs], usb[0:64, fs],
                                             bcb[:, fs])
                        nc.sync.dma_start(atp[chunk][m][64:128, fs],
                                          stg[:, fs])
                if chunk == 0 and h == 7:
                    r = 68
                    for mt in range(8):
                        for qc in range(2):
                            bg.append((int(r),
                                       (lambda mt=mt, qc=qc: oproj(mt, qc))))
                            r += 3.5

            # ---- main loop: one global software pipeline over all units ----
            # A unit covers TWO ktiles of one head: the sc/ptt ring reuse
            # latencies (exp + semaphores) amortize under ~2.6us of matmuls.
            # chunk-1 heads run odd-first so the final head's normalization
            # is an even (direct-write) one: shortest possible tail chain.
            LA = 2  # score -> PV lookahead (double-units)
            h_order1 = [1, 3, 5, 7, 0, 2, 4, 6]
            units = ([(0, h, 2 * j) for h in range(H) for j in range(NK // 2)]
                     + [(1, h, 2 * j) for h in h_order1
                        for j in range(NK // 2)])
            NU = len(units)
            accs = {}
            ring = {}
            for u in range(NU + LA):
                if u < NU:
                    c, h, k0_ = units[u]
                    m, rh = h // 2, 64 * (h % 2)
                    q0 = c * CH
                    ptts = []
                    for k in (k0_, k0_ + 1):
                        sc = ps.tile([P, CH], F32, tag="sc", bufs=2, name="sc")
                        for half in range(2):
                            nc.tensor.matmul(
                                sc[:, half * 512:(half + 1) * 512],
                                kt[m][rh:rh + 64, k * P:(k + 1) * P],
                                qt[m][rh:rh + 64,
                                      q0 + half * 512:q0 + (half + 1) * 512],
                                start=True, stop=True)
                        ptt = ptp.tile([P, CH], BF16, tag="pt",
                                       bufs=2 * LA + 2, name="pt")
                        nc.scalar.activation(ptt[:], sc[:], Exp)
                        ptts.append(ptt)
                    ring[u] = ptts
                drain_tick(u)
                if u >= LA:
                    c, h, k0_ = units[u - LA]
                    if k0_ == 0:
                        accs[(c, h)] = ps.tile([P, CH], F32, tag="acc",
                                               bufs=1, name="acc")
                    acc = accs[(c, h)]
                    ptts_p = ring.pop(u - LA)
                    for i, k in enumerate((k0_, k0_ + 1)):
                        for qb in range(2):
                            nc.tensor.matmul(
                                acc[0:65, qb * 512:(qb + 1) * 512],
                                vt[k][:, h, 0:65],
                                ptts_p[i][:, qb * 512:(qb + 1) * 512],
                                start=(k == 0), stop=(k == NK - 1))
                    if k0_ + 1 == NK - 1:
                        norm_head(c, h, accs.pop((c, h)))

            drain_all()
            # chunk-1 out-projection (tail); qc-major so the first half of
            # the last head's normalization unblocks 8 chunks immediately
            for qc in range(2, 4):
                for mt in range(8):
                    oproj(mt, qc)

    nc.compile()
    return nc


def _get_nc():
    global _CACHED_NC
    if _CACHED_NC is None:
        _CACHED_NC = _emit_kernel()
    return _CACHED_NC


def kernel(inputs, context, Wq, Wk, Wv, Wo, bo, **kw):
    global LAST_RESULT
    scale = DH ** -0.5
    bf = ml_dtypes.bfloat16
    wq_s = (np.asarray(Wq, np.float32) * scale).astype(bf)
    wk_s = np.asarray(Wk, np.float32).astype(bf)
    wv_s = np.asarray(Wv, np.float32).astype(bf)
    wo_s = np.asarray(Wo, np.float32).astype(bf)

    in_maps = []
    for c in range(N_CORES):
        b, g = c // 2, c % 2
        sl = slice(g * DG, (g + 1) * DG)
        in_maps.append({
            "xT": np.ascontiguousarray(np.asarray(inputs[b], np.float32).T).astype(bf),
            "cT": np.ascontiguousarray(np.asarray(context[b], np.float32).T).astype(bf),
            "wq": np.ascontiguousarray(wq_s[:, sl]),
            "wk": np.ascontiguousarray(wk_s[:, sl]),
            "wv": np.ascontiguousarray(wv_s[:, sl]),
            "wo": np.ascontiguousarray(wo_s[sl, :]),
        })

    nc = _get_nc()
    res = run_bass_kernel_spmd(nc, in_maps, core_ids=list(range(N_CORES)))
    LAST_RESULT = res

    out = np.empty((B, S, D), np.float32)
    bo32 = np.asarray(bo, np.float32)
    for b in range(B):
        out[b] = (res.results[2 * b]["outT"].astype(np.float32)
                  + res.results[2 * b + 1]["outT"].astype(np.float32)).T + bo32
    return out
